# revision 10
# baseline (speedup 1.0000x reference)
"""Trainium2 Bass/Tile kernel for ExtAttentionPool (nn_ExtAttentionPool).

Math (per sample b):
    S[i, o]  = sum_d L[i, d] * W[o, d]
    E[o, i]  = exp(S[i,o]/O)            (bias cancels in the softmax over i)
    Z[o]     = sum_i E[o, i]
    OUT[o,t] = (1/Z[o]) * sum_i E[o, i] * L[t, i]
    result row b = OUT flattened (O-major), shape (O*T,)

Sharding: data-parallel over batch B=16 across 8 cores (2 samples/core).

Implementation:
  - logits are cast to bf16 AND transposed/swizzled on the host into
    y[kp, p, s, c, t] with d = 128c+p, t_global = TB*kp + t.  Both matmuls
    contract over logits' d axis, so the device needs Lt (d on partitions);
    doing the transpose host-side removes all on-chip transpose matmuls and
    the PSUM->SBUF copy traffic they require.
  - The per-core load is NKP contiguous 1 MiB DMA chunks (chunk kp = all
    data for t-block kp of both samples), issued up front on the sync
    HWDGE queue; large transfers run near the ~358 GB/s per-core HBM cap.
  - The two samples are packed side by side in PE column groups: sample 0
    writes PSUM partitions 0:10, sample 1 partitions 32:42
    (tile_position=(0,32)).  Packed matmul pairs stream concurrently, so
    mm1+mm2 for both samples cost barely more than for one.
  - mm1 for t-block kp runs as soon as chunk kp lands (contraction data
    for a t-block arrives together).  exp (with Z row-sum accumulated by
    the activation) and the tiny E-transpose (identity matmul) follow, and
    mm2 is an L-shaped (j, m) grid: column m=kp as chunk kp lands, rows
    j=2kp,2kp+1 once ec blocks exist.  Only ~14 matmul pairs + one exp
    remain after the last byte arrives.
  - 1/Z rides the PSUM->SBUF copies at the end (ScalarE/DVE alternating),
    one output DMA per (sample, t-block) on the two HWDGE queues.
"""

import numpy as np
import ml_dtypes
from contextlib import ExitStack

_np_bf16 = ml_dtypes.bfloat16

import concourse.bass as bass
import concourse.mybir as mybir
import concourse.tile as tile
from concourse import bacc
from concourse.bass_utils import run_bass_kernel_spmd
from concourse.masks import make_identity

F32 = mybir.dt.float32
BF16 = mybir.dt.bfloat16

N_CORES = 8
B_FULL = 16
P = 128
T = 1024
D = 1024
O = 10
NKP = 4              # DMA chunks / t-super-blocks per core
TB = T // NKP        # 256: t-columns per chunk
ND = D // P          # 8 contraction steps
NJ = T // P          # 8 E-transpose blocks
SOFF = 32            # partition strip offset per sample (col group)


def build_nc(b_per=2, warmup_mms=42):
    """Per-core Bass program (bf16 compute, both samples packed)."""
    nc = bacc.Bacc(
        "TRN2", target_bir_lowering=False, debug=False, enable_asserts=False
    )
    y = nc.dram_tensor("y", (NKP, P, b_per, ND, TB), BF16, kind="ExternalInput").ap()
    wt_in = nc.dram_tensor("wt", (P, ND, O), BF16, kind="ExternalInput").ap()
    out = nc.dram_tensor("out", (b_per, O * T), F32, kind="ExternalOutput").ap()

    with tile.TileContext(nc) as tc, ExitStack() as ctx:
        singles = ctx.enter_context(tc.tile_pool(name="singles", bufs=1))
        sc_ps = ctx.enter_context(tc.tile_pool(name="sc", bufs=1, space="PSUM"))
        o_ps = ctx.enter_context(tc.tile_pool(name="o", bufs=1, space="PSUM"))
        et_ps = ctx.enter_context(tc.tile_pool(name="et", bufs=2, space="PSUM"))

        # --- the whole load: NKP 1 MiB chunks, in order, on the sync
        # ring.  Monolithic 1 MiB transfers stream gaplessly at ~400 GB/s;
        # finer splits get paced by the ~0.6-0.8us per-DMA issue cost.
        lt = singles.tile([P, NKP, b_per, ND, TB], BF16)
        for kp in range(NKP):
            nc.sync.dma_start(out=lt[:, kp], in_=y[kp])

        # small inputs ride the other HWDGE ring
        wt_sb = singles.tile([P, ND, O], BF16)
        nc.scalar.dma_start(out=wt_sb, in_=wt_in)

        # scores / mm2-out PSUM: [42, 512] banks, strips per sample
        sc_t = [sc_ps.tile([SOFF + O, 2 * TB], F32, name=f"sc{h}") for h in range(2)]
        # one PSUM bank per output t-block: a start=True matmul clears the
        # has_written bits for its whole (bank x partition-row), so no two
        # concurrently-open accumulation groups may share bank+partitions.
        o_t = [o_ps.tile([P, 2 * TB], F32, name=f"ot{m}") for m in range(NKP)]

        # --- PE warmup: >=3.41us of back-to-back matmuls flips the HAM
        # clock gate to 2.4 GHz while chunk 0 is still streaming in.  A
        # zeroed tile is ready ~1.3us before make_identity, so warm on that
        # and build the identity (needed only by the E-transpose) after.
        wsrc = singles.tile([P, 4 * P], BF16)
        nc.gpsimd.memset(wsrc, 0.0)
        for i in range(warmup_mms):
            nc.tensor.matmul(
                o_t[0][64:P, 0:P], lhsT=wsrc[:, 0:64], rhs=wsrc[:, 0:P],
                start=True, stop=True, tile_position=(0, 64),
            )

        ident = singles.tile([P, P], BF16)
        make_identity(nc, ident)

        e_sb = singles.tile([SOFF + O, T], BF16)
        ec = singles.tile([P, b_per, NJ, O], BF16)
        zparts = singles.tile([SOFF + O, NKP], F32)
        o_sb = singles.tile([SOFF + O, T], F32)  # scaled output staging

        def strip(ap2d, s, cols):
            return ap2d[SOFF * s : SOFF * s + O, cols]

        def sc_loc(kp):
            return sc_t[kp // 2], slice((kp % 2) * TB, (kp % 2) * TB + TB)

        def mm2_pair(j, m, s):
            nc.tensor.matmul(
                strip(o_t[m], s, slice(0, TB)),
                lhsT=ec[:, s, j, :],
                rhs=lt[:, m, s, j, :],
                start=(j == 0),
                stop=(j == NJ - 1),
                tile_position=(0, SOFF * s),
            )

        out2d = [out[s].rearrange("(o t) -> o t", o=O) for s in range(b_per)]

        for kp in range(NKP):
            sct, cols = sc_loc(kp)
            # mm1 for t-block kp (both samples packed per contraction step)
            for c in range(ND):
                for s in range(b_per):
                    nc.tensor.matmul(
                        strip(sct, s, cols),
                        lhsT=wt_sb[:, c, :],
                        rhs=lt[:, kp, s, c, :],
                        start=(c == 0),
                        stop=(c == ND - 1),
                        tile_position=(0, SOFF * s),
                    )
            # mm2 column m=kp for all ec blocks already available
            for j in range(2 * kp):
                for s in range(b_per):
                    mm2_pair(j, kp, s)
            if kp == 0:
                # bridge the exp(kp0) fence, keep the busy span contiguous
                for _ in range(4):
                    nc.tensor.matmul(
                        o_t[0][64:P, 0:P], lhsT=wsrc[:, 0:64],
                        rhs=wsrc[:, 0:P], start=True, stop=True,
                        tile_position=(0, 64),
                    )
            # exp with Z row-sum accumulation (one op covers both sample
            # strips; rows 10:32 are garbage and never read).  The last
            # chunk's exp is split in two and its Z part moved to DVE so the
            # E-transpose can start as early as possible.
            ecols = slice(kp * TB, (kp + 1) * TB)
            if kp < NKP - 1:
                nc.scalar.activation(
                    out=e_sb[:, ecols],
                    in_=sct[:, cols],
                    func=mybir.ActivationFunctionType.Exp,
                    scale=1.0 / O,
                    accum_out=zparts[:, kp : kp + 1],
                )
            else:
                for h in range(2):
                    hcol = slice(kp * TB + h * P, kp * TB + (h + 1) * P)
                    nc.scalar.activation(
                        out=e_sb[:, hcol],
                        in_=sct[:, slice(cols.start + h * P, cols.start + (h + 1) * P)],
                        func=mybir.ActivationFunctionType.Exp,
                        scale=1.0 / O,
                    )
                nc.vector.reduce_sum(
                    zparts[:, kp : kp + 1], e_sb[:, ecols],
                    axis=mybir.AxisListType.X,
                )
            # E-transpose for the two fresh 128-blocks
            for j in (2 * kp, 2 * kp + 1):
                for s in range(b_per):
                    etp = et_ps.tile([P, O], F32, tag="et", name=f"et{kp}_{j}_{s}")
                    nc.tensor.matmul(
                        etp,
                        lhsT=e_sb[SOFF * s : SOFF * s + O, j * P : (j + 1) * P],
                        rhs=ident[SOFF * s : SOFF * s + O, SOFF * s : SOFF * s + O],
                        start=True, stop=True,
                    )
                    nc.vector.tensor_copy(ec[:, s, j, :], etp)
            # mm2 rows j=2kp, 2kp+1 for columns m<=kp (m-major so early
            # columns stop first and their scale/store can begin)
            for m in range(kp + 1):
                for j in (2 * kp, 2 * kp + 1):
                    for s in range(b_per):
                        mm2_pair(j, m, s)

        # softmax denominator per strip
        zsum = singles.tile([SOFF + O, 1], F32)
        nc.vector.reduce_sum(zsum, zparts, axis=mybir.AxisListType.X)
        rz = singles.tile([SOFF + O, 1], F32)
        nc.vector.reciprocal(rz, zsum)

        # scale by 1/Z on the PSUM->SBUF copy: one [42, TB] op per output
        # t-block (bank), ScalarE on banks 0/1, DVE on banks 2/3 so the two
        # engines never contend for the same PSUM bank.
        for m in range(NKP):
            dcols = slice(m * TB, (m + 1) * TB)
            if m < 2:
                nc.scalar.activation(
                    out=o_sb[0:SOFF + O, dcols], in_=o_t[m][0:SOFF + O, 0:TB],
                    func=mybir.ActivationFunctionType.Copy, scale=rz,
                )
            else:
                nc.vector.tensor_scalar_mul(
                    o_sb[0:SOFF + O, dcols], o_t[m][0:SOFF + O, 0:TB], rz
                )
            if m % 2 == 1:  # halves 0:512 / 512:1024 complete -> store
                hcols = slice((m - 1) * TB, (m + 1) * TB)
                for s in range(b_per):
                    eng = nc.sync if s == 0 else (nc.gpsimd if m == 1 else nc.scalar)
                    eng.dma_start(
                        out=out2d[s][:, hcols], in_=strip(o_sb, s, hcols)
                    )

    nc.compile()
    return nc


_NC = None
TRACE = False
LAST_RESULT = None
BUILD_KWARGS = {}


def _get_nc():
    global _NC
    if _NC is None:
        _NC = build_nc(**BUILD_KWARGS)
    return _NC


def kernel(logits, decision, W, b):
    """Full-input entry point: shards batch over 8 cores, returns (16, 10240)."""
    global LAST_RESULT
    lg = np.asarray(logits, dtype=np.float32).astype(_np_bf16)
    Od, Dd = W.shape
    # wt[p, c, o] = W[o, 128c + p]  (host-side transpose of the tiny weight)
    wt = np.ascontiguousarray(
        np.asarray(W, dtype=np.float32).T
        .reshape(Dd // P, P, Od)
        .transpose(1, 0, 2)
    ).astype(_np_bf16)
    nc = _get_nc()
    bp = B_FULL // N_CORES
    in_maps = []
    for i in range(N_CORES):
        pair = lg[i * bp : (i + 1) * bp]  # (2, T, D)
        # y[kp, p, s, c, t] = pair[s, TB*kp + t, 128c + p]
        yarr = np.ascontiguousarray(
            pair.reshape(bp, NKP, TB, ND, P).transpose(1, 4, 0, 3, 2)
        )
        in_maps.append({"y": yarr, "wt": wt})
    res = run_bass_kernel_spmd(nc, in_maps, core_ids=list(range(N_CORES)), trace=TRACE)
    LAST_RESULT = res
    return np.concatenate([res.results[i]["out"] for i in range(N_CORES)], axis=0)


# revision 12
# speedup vs baseline: 1.0483x; 1.0483x over previous
"""Trainium2 Bass/Tile kernel for ExtAttentionPool — 4-way column-group packed.

Same math/dataflow as kernel.py, but every matmul phase packs FOUR
concurrent streams into PE column groups (tile_position (0,0/32/64/96)):
  - mm1: (sample, t-half) -> strips 0/32/64/96, N=128 each
  - mm2 rows: (sample, m-parity) -> strips, N=256 each
so the PE wall time for the two big matmuls halves again vs 2-way packing.
Scores use ONE PSUM bank (chunk kp at cols kp*128, halves at different
partition strips); mm2 uses two banks (bank m//2, partitions by m%2) so no
two open accumulation groups share bank+partitions (the has_written
partition-row rule).  Z needs a cross-partition combine (strip h0 + strip
h1): a host-provided 128x128 selection matrix M (ones at k-p in {-64,0,64})
does it in one f32 matmul: rz_all = 1 / (M.T @ zsum).
"""

import numpy as np
import ml_dtypes
from contextlib import ExitStack

_np_bf16 = ml_dtypes.bfloat16

import concourse.bass as bass
import concourse.mybir as mybir
import concourse.tile as tile
from concourse import bacc
from concourse.bass_utils import run_bass_kernel_spmd
from concourse.masks import make_identity

F32 = mybir.dt.float32
BF16 = mybir.dt.bfloat16

N_CORES = 8
B_FULL = 16
P = 128
T = 1024
D = 1024
O = 10
NKP = 4
TB = T // NKP        # 256
HB = TB // 2         # 128: mm1 half-block
ND = D // P
NJ = T // P
SOFF = 32


def build_nc(b_per=2, warmup_mms=40):
    nc = bacc.Bacc(
        "TRN2", target_bir_lowering=False, debug=False, enable_asserts=False
    )
    y = nc.dram_tensor("y", (NKP, P, b_per, ND, TB), BF16, kind="ExternalInput").ap()
    wt_in = nc.dram_tensor("wt", (P, ND, O), BF16, kind="ExternalInput").ap()
    sel_in = nc.dram_tensor("sel", (P, P), F32, kind="ExternalInput").ap()
    out = nc.dram_tensor("out", (b_per, O * T), F32, kind="ExternalOutput").ap()

    with tile.TileContext(nc) as tc, ExitStack() as ctx:
        singles = ctx.enter_context(tc.tile_pool(name="singles", bufs=1))
        sc_ps = ctx.enter_context(tc.tile_pool(name="sc", bufs=1, space="PSUM"))
        o_ps = ctx.enter_context(tc.tile_pool(name="o", bufs=1, space="PSUM"))
        et_ps = ctx.enter_context(tc.tile_pool(name="et", bufs=2, space="PSUM"))
        w_ps = ctx.enter_context(tc.tile_pool(name="wp", bufs=1, space="PSUM"))

        lt = singles.tile([P, NKP, b_per, ND, TB], BF16)
        for kp in range(NKP):
            nc.sync.dma_start(out=lt[:, kp], in_=y[kp])

        wt_sb = singles.tile([P, ND, O], BF16)
        nc.scalar.dma_start(out=wt_sb, in_=wt_in)
        sel_sb = singles.tile([P, P], F32)
        nc.scalar.dma_start(out=sel_sb, in_=sel_in)

        sc = sc_ps.tile([P, NKP * HB], F32, name="sc")       # 1 bank
        o_t = [o_ps.tile([P, 2 * TB], F32, name=f"ot{h}") for h in range(2)]
        warm = w_ps.tile([P, 4 * P], F32, name="warm")

        ident = singles.tile([P, P], BF16)
        make_identity(nc, ident)

        def warm_mm():
            nc.tensor.matmul(
                warm[:, 0:P], lhsT=ident, rhs=ident,
                start=True, stop=True,
            )

        for i in range(warmup_mms):
            warm_mm()

        e_sb = singles.tile([P, T], BF16)
        ec = singles.tile([P, b_per, NJ, O], BF16)
        zparts = singles.tile([P, 2 * NKP], F32)
        nc.gpsimd.memset(zparts, 0.0)
        o_sb = singles.tile([P, T], F32)

        def g1off(s, h):          # mm1/scores strip offset
            return 64 * h + SOFF * s

        def g2off(s, m):          # mm2 strip offset
            return 64 * (m % 2) + SOFF * s

        def mm2_one(j, m, s):
            r = g2off(s, m)
            nc.tensor.matmul(
                o_t[m // 2][r : r + O, 0:TB],
                lhsT=ec[:, s, j, :],
                rhs=lt[:, m, s, j, :],
                start=(j == 0),
                stop=(j == NJ - 1),
                tile_position=(0, r),
            )

        out2d = [out[s].rearrange("(o t) -> o t", o=O) for s in range(b_per)]

        for kp in range(NKP):
            ccols = slice(kp * HB, (kp + 1) * HB)
            # mm1: 4-way (sample x t-half) quads per contraction step
            for c in range(ND):
                for h in range(2):
                    for s in range(b_per):
                        r = g1off(s, h)
                        nc.tensor.matmul(
                            sc[r : r + O, ccols],
                            lhsT=wt_sb[:, c, :],
                            rhs=lt[:, kp, s, c, h * HB : (h + 1) * HB],
                            start=(c == 0),
                            stop=(c == ND - 1),
                            tile_position=(0, r),
                        )
            # mm2 column m=kp for available ec blocks (2-way pairs)
            for j in range(2 * kp):
                for s in range(b_per):
                    mm2_one(j, kp, s)
            if kp == 0:
                for _ in range(6):
                    warm_mm()
            # exp per t-half (both its sample strips in one op)
            for h in range(2):
                j = 2 * kp + h
                rr = slice(64 * h, 64 * h + SOFF + O)
                nc.scalar.activation(
                    out=e_sb[rr, j * P : (j + 1) * P],
                    in_=sc[rr, ccols],
                    func=mybir.ActivationFunctionType.Exp,
                    scale=1.0 / O,
                    accum_out=zparts[rr, j : j + 1],
                )
            # E-transpose
            for h in range(2):
                j = 2 * kp + h
                for s in range(b_per):
                    r = g1off(s, h)
                    etp = et_ps.tile([P, O], F32, tag="et", name=f"et{j}_{s}")
                    nc.tensor.matmul(
                        etp,
                        lhsT=e_sb[r : r + O, j * P : (j + 1) * P],
                        rhs=ident[r : r + O, r : r + O],
                        start=True, stop=True,
                    )
                    nc.vector.tensor_copy(ec[:, s, j, :], etp)
            # mm2 rows: 4-way (sample x m-parity) quads
            for mp in range(0, kp + 1, 2):
                ms = [m for m in (mp, mp + 1) if m <= kp]
                for j in (2 * kp, 2 * kp + 1):
                    for m in ms:
                        for s in range(b_per):
                            mm2_one(j, m, s)
            # HAM duty fillers (this version is DMA-paced, so real work
            # alone leaves the PE idle enough to re-throttle)
            if kp < NKP - 1:
                for _ in range((6, 6, 8)[kp]):
                    warm_mm()

        # Z: per-strip sums, then cross-strip combine via the selection
        # matrix (rep[p] = sum of zsum at p-64, p, p+64 -> Z(sample) at
        # every strip position), then reciprocal
        zsum = singles.tile([P, 1], F32)
        nc.vector.reduce_sum(zsum, zparts, axis=mybir.AxisListType.X)
        rep = et_ps.tile([P, 1], F32, tag="et", name="zrep")
        nc.tensor.matmul(rep, lhsT=sel_sb, rhs=zsum, start=True, stop=True)
        rz_all = singles.tile([P, 1], F32)
        nc.vector.reciprocal(rz_all, rep)

        # scale by 1/Z on PSUM->SBUF copy; ScalarE bank0 (m0,m1), DVE bank1
        for m in range(NKP):
            rr = slice(64 * (m % 2), 64 * (m % 2) + SOFF + O)
            dst = o_sb[rr, m * TB : (m + 1) * TB]
            src = o_t[m // 2][rr, 0:TB]
            if m // 2 == 0:
                nc.scalar.activation(
                    out=dst, in_=src,
                    func=mybir.ActivationFunctionType.Copy, scale=rz_all[rr, :],
                )
            else:
                nc.vector.tensor_scalar_mul(dst, src, rz_all[rr, :])

        # stores: per (sample, m-parity): [10, 2, 256] strided pieces
        o_sb4 = o_sb.rearrange("p (a c) -> p a c", c=2 * TB)
        for par in range(2):
            for s in range(b_per):
                r = 64 * par + SOFF * s
                dst = out2d[s].rearrange("o (a c) -> o a c", c=2 * TB)[
                    :, :, par * TB : (par + 1) * TB
                ]
                eng = nc.sync if s == 0 else nc.scalar
                eng.dma_start(
                    out=dst,
                    in_=o_sb4[r : r + O, :, par * TB : (par + 1) * TB],
                )

    nc.compile()
    return nc


_NC = None
TRACE = False
LAST_RESULT = None
BUILD_KWARGS = {}


def _get_nc():
    global _NC
    if _NC is None:
        _NC = build_nc(**BUILD_KWARGS)
    return _NC


def kernel(logits, decision, W, b):
    global LAST_RESULT
    lg = np.asarray(logits, dtype=np.float32).astype(_np_bf16)
    Od, Dd = W.shape
    wt = np.ascontiguousarray(
        np.asarray(W, dtype=np.float32).T
        .reshape(Dd // P, P, Od)
        .transpose(1, 0, 2)
    ).astype(_np_bf16)
    k = np.arange(P)
    sel = ((np.abs(k[:, None] - k[None, :]) % 64 == 0)
           & (np.abs(k[:, None] - k[None, :]) <= 64)).astype(np.float32)
    nc = _get_nc()
    bp = B_FULL // N_CORES
    in_maps = []
    for i in range(N_CORES):
        pair = lg[i * bp : (i + 1) * bp]
        yarr = np.ascontiguousarray(
            pair.reshape(bp, NKP, TB, ND, P).transpose(1, 4, 0, 3, 2)
        )
        in_maps.append({"y": yarr, "wt": wt, "sel": sel})
    res = run_bass_kernel_spmd(nc, in_maps, core_ids=list(range(N_CORES)), trace=TRACE)
    LAST_RESULT = res
    return np.concatenate([res.results[i]["out"] for i in range(N_CORES)], axis=0)
